# revision 46
# baseline (speedup 1.0000x reference)
"""AR(128) prediction + MSE/L1 loss on 8 Trainium2 NeuronCores.

pred[i] = sum_j params[j] * y[i+j]  (i = 0..1_999_999)  -- a 128-tap FIR.
loss    = mean((pred - y[128:])**2) + sum(|params|)

Strategy (per the row-shard + halo hint):
  * The series is split into 8 row-shards with a 128-sample halo.
    Core s computes preds [s*249984, s*249984 + 250112) (multiple-of-128
    shards that overlap by 128 so every core runs an identical program).
  * On device the FIR is phase-decomposed: with Yt[k, C] = y[128C + k],
      pred[128C + m] = sum_k W0[k, m] Yt[k, C] + sum_k W1[k, m] Yt[k, C+1]
    where W0[k, m] = params[k - m] (k >= m), W1[k, m] = params[k+128-m]
    (k < m).  W0/W1 are pure re-indexings of params (host as_strided,
    zero arithmetic), replicated to all cores.
  * y is DMA'd in 4 contiguous [128, F] chunks, transposed on-chip
    128x128-tile-wise by the tensor engine (chunk c covers exactly yt
    columns [512c, 512c+F)), the two banded matmuls accumulate bank c
    in PSUM, and the phase-major result is transposed back per chunk
    before contiguous [128, F] DMAs out.
  * The squared-error partial is computed per PSUM bank: the vector
    engine forms d = pred - y+ straight off PSUM, the scalar engine
    squares with a fused row-accumulate, and a ones-matmul
    partition-reduces.  Each core emits
    partial = sum(d^2)/2e6 + sum(|params|)/8; the host sums the 8
    scalars (a device AllReduce has a ~20us latency floor).
  * The conv matmuls run as float32r (single-pass reduced-precision
    fp32, ~1.5e-4 rel err vs the fp32 reference); set KERNEL_NO_F32R=1
    for exact fp32 (4 cycles/row instead of 1-2).
  * Dummy matmuls during the DMA-wait window plus real matmuls woven
    between the transpose groups keep the PE HAM activity monitor fed
    so the kernel runs at 2.4 GHz instead of the cold 1.2 GHz clock.
"""

import sys

if "/opt/trn_rl_repo" not in sys.path:
    sys.path.insert(0, "/opt/trn_rl_repo")

import numpy as np

P = 128
N_FULL = 2_000_128
N_PRED = 2_000_000
N_CORES = 8
OWN = 249_984          # preds owned by cores 0..6           (= 1953*128)
SHARD_PRED = 250_112   # preds computed per core             (= 1954*128)
SHARD_Y = 250_240      # y samples per core incl. halo       (= 1955*128)
NCOLS = 1954           # phase-major pred columns
YT_COLS = 1955         # transposed-y columns
INV_N = 1.0 / float(N_PRED)

# chunk table: (y/pred element offset, partition width F, 128x128 tiles)
CHUNKS = [(0, 512, 4), (65536, 512, 4), (131072, 512, 4), (196608, 384, 3)]

LAST_EXEC_NS = None
LAST_RESULTS = None

_CACHE = {}


def _build_nc(use_f32r=False):
    import concourse.bacc as bacc
    import concourse.tile as tile
    from concourse import mybir
    from concourse.masks import make_identity

    f32 = mybir.dt.float32
    f32r = mybir.dt.float32r
    AF = mybir.ActivationFunctionType
    ALU = mybir.AluOpType

    nc = bacc.Bacc("TRN2", target_bir_lowering=False, debug=False)

    y = nc.declare_dram_parameter("y", [SHARD_Y], f32, False)
    w0 = nc.declare_dram_parameter("w0", [128, 128], f32, False)
    w1 = nc.declare_dram_parameter("w1", [128, 128], f32, False)
    pp = nc.declare_dram_parameter("pp", [1, 128], f32, False)
    fl = nc.declare_dram_parameter("fl", [1, 1], f32, False)
    idin = nc.declare_dram_parameter("idin", [128, 128], f32, False)
    pred = nc.declare_dram_parameter("pred", [SHARD_PRED], f32, True)
    lp = nc.declare_dram_parameter("lp", [1, 1], f32, True)

    def mm_ap(ap):
        return ap.bitcast(f32r) if use_f32r else ap

    mm_dt = None  # set inside builder

    with tile.TileContext(nc) as tc:
        with (
            tc.tile_pool(name="const", bufs=1) as constp,
            tc.tile_pool(name="big", bufs=1) as bigp,
            tc.tile_pool(name="nbp", bufs=2) as nbp,
            tc.tile_pool(name="osbp", bufs=2) as osbp,
            tc.tile_pool(name="tps", bufs=3, space="PSUM") as tpsp,
            tc.tile_pool(name="mps", bufs=4, space="PSUM") as mpsp,
            tc.tile_pool(name="rps", bufs=1, space="PSUM") as rpsp,
        ):
            # ---- constants -------------------------------------------------
            w0_sb = constp.tile([128, 128], f32)
            w1_sb = constp.tile([128, 128], f32)
            pp_sb = constp.tile([1, 128], f32)
            fl_sb = constp.tile([1, 1], f32)
            mm_dt = f32r if use_f32r else f32
            ident = constp.tile([128, 128], mm_dt)
            ones = constp.tile([128, 1], f32)
            nc.gpsimd.memset(ones[:], 1.0)
            nc.scalar.dma_start(mm_ap(w0_sb[:]), mm_ap(w0[:, :]))
            nc.scalar.dma_start(ident[:], mm_ap(idin[:, :]))
            nc.scalar.dma_start(mm_ap(w1_sb[:]), mm_ap(w1[:, :]))

            yt = bigp.tile([128, YT_COLS], f32)
            nbtail = bigp.tile([35, 128], mm_dt)

            # rstack cols: 0-3 sum(d^2) main cols per bank, 4 tail col 1953
            rstack = constp.tile([128, 5], f32)
            df = bigp.tile([128, NCOLS], f32)
            sq = bigp.tile([128, NCOLS], f32)
            pred_sb = bigp.tile([128, NCOLS], f32)

            # HAM warm-up: the PE clock idles at 1.2 GHz and only ramps to
            # 2.4 GHz after ~3.4 us of sustained non-transpose matmul
            # activity.  Burn dummy matmuls on the identity during the
            # DMA-wait window so the real work runs at full clock.
            dumw = constp.tile([128, 128], f32)
            nc.vector.memset(dumw[:], 0.0)
            wps = rpsp.tile([128, 128], f32, tag="rp")
            for _ in range(8):
                nc.tensor.matmul(
                    wps[:], dumw[:], dumw[:], start=True, stop=True
                )

            ppsb = [None] * 4

            def emit_chunk(c):
                off, F, nt = CHUNKS[c]
                nbc = nbp.tile([128, 512], mm_dt, tag="nb")
                dma_eng = nc.sync if c % 2 == 0 else nc.scalar
                dma_eng.dma_start(
                    nbc[:, 0:F],
                    mm_ap(y[off : off + 128 * F].rearrange("(p f) -> p f", f=F)),
                )
                if c == 2:
                    nc.sync.dma_start(
                        nbtail[:],
                        mm_ap(y[245760:250240].rearrange("(p f) -> p f", f=128)),
                    )
                    nc.sync.dma_start(pp_sb[:], pp[:, :])
                    nc.sync.dma_start(fl_sb[:], fl[:, :])
                pst = tpsp.tile([128, 512], f32r if use_f32r else f32, tag="tps", name=f"pst{c}")
                for t in range(nt):
                    nc.tensor.transpose(
                        pst[:, 128 * t : 128 * (t + 1)],
                        nbc[:, 128 * t : 128 * (t + 1)],
                        ident[:],
                    )
                if c == 3:
                    nc.tensor.transpose(
                        pst[:, 384:419], nbtail[:], ident[0:35, 0:35]
                    )
                # pst[k, 128t + p] -> yt[k, off/128 + nt*p + t]
                ytv = yt[:, off // 128 : off // 128 + F].rearrange(
                    "k (p t) -> k p t", t=nt
                )
                psv = pst[:, 0 : 128 * nt].rearrange("k (t p) -> k p t", p=128)
                nc.vector.tensor_copy(mm_ap(ytv), psv)
                if c == 3:
                    nc.vector.tensor_copy(
                        mm_ap(yt[:, 1920:1955]), pst[:, 384:419]
                    )

            def emit_conv(b, h):
                # half-bank matmul pair: cols [256h, 256h+wh) of bank b
                c0 = 512 * b + 256 * h
                wb = 512 if b < 3 else NCOLS - 1536
                wh = min(256, wb - 256 * h)
                if h == 0:
                    ppsb[b] = mpsp.tile(
                        [128, 512], f32, tag="mm", name=f"ppsb{b}"
                    )
                o0 = 256 * h
                nc.tensor.matmul(
                    ppsb[b][:, o0 : o0 + wh],
                    mm_ap(w0_sb[:]),
                    mm_ap(yt[:, c0 : c0 + wh]),
                    start=True,
                    stop=False,
                )
                nc.tensor.matmul(
                    ppsb[b][:, o0 : o0 + wh],
                    mm_ap(w1_sb[:]),
                    mm_ap(yt[:, c0 + 1 : c0 + 1 + wh]),
                    start=False,
                    stop=True,
                )

            def emit_predh(hb):
                # copy one conv half-bank psum -> pred_sb (phase-major)
                b, h = hb // 2, hb % 2
                wb = 512 if b < 3 else NCOLS - 1536
                wh = min(256, wb - 256 * h)
                c0 = 512 * b + 256 * h
                nc.vector.tensor_copy(
                    pred_sb[:, c0 : c0 + wh], ppsb[b][:, 256 * h : 256 * h + wh]
                )

            def emit_loss(b):
                # DVE: diff; ACT: square + row-accum
                c0 = 512 * b
                w = 512 if b < 3 else NCOLS - 1536
                nc.vector.tensor_sub(
                    df[:, c0 : c0 + w],
                    ppsb[b][:, 0:w],
                    yt[:, c0 + 1 : c0 + 1 + w],
                )
                wm = w if b < 3 else w - 1  # main cols exclude col 1953
                nc.scalar.activation(
                    sq[:, c0 : c0 + wm],
                    df[:, c0 : c0 + wm],
                    AF.Square,
                    accum_out=rstack[:, b : b + 1],
                )
                if b == 3:
                    nc.scalar.activation(
                        sq[:, 1953:1954],
                        df[:, 1953:1954],
                        AF.Square,
                        accum_out=rstack[:, 4:5],
                    )

            def emit_out(hb):
                # transpose one half-bank of pred back to natural + DMA out.
                # out-chunk hb covers pred cols [256*hb, 256*hb + F).
                F = 256 if hb < 7 else 128
                nt = F // 128
                off = 32768 * hb
                c0 = 256 * hb
                ops = tpsp.tile([128, 512], f32, tag="tps")
                pmv = pred_sb[:, c0 : c0 + F].rearrange("m (p t) -> m p t", t=nt)
                for t in range(nt):
                    nc.tensor.transpose(
                        ops[:, 128 * t : 128 * (t + 1)], pmv[:, :, t], ident[:]
                    )
                if hb == 7:
                    nc.tensor.transpose(
                        ops[0:34, 128:256], pred_sb[:, 1920:1954], ident[:]
                    )
                osb = osbp.tile([128, 512], f32, tag="osb")
                if hb % 2 == 0:
                    nc.scalar.copy(osb[:, 0:F], ops[:, 0:F])
                else:
                    nc.vector.tensor_copy(osb[:, 0:F], ops[:, 0:F])
                dma_eng = nc.sync if hb % 2 == 0 else nc.scalar
                dma_eng.dma_start(
                    pred[off : off + 128 * F].rearrange("(p f) -> p f", f=F),
                    osb[:, 0:F],
                )
                if hb == 7:
                    nc.vector.tensor_copy(osb[0:34, 128:256], ops[0:34, 128:256])
                    nc.sync.dma_start(
                        pred[245760:250112].rearrange("(p f) -> p f", f=128),
                        osb[0:34, 128:256],
                    )

            # pipelined emission; conv half (b,0) needs chunk b, (b,1) needs
            # chunks b and b+1 (halo column).  Real matmuls and output
            # transposes woven between the transpose groups keep the HAM
            # activity monitor fed and spread the output DMAs early.
            emit_chunk(0)
            emit_conv(0, 0)
            emit_chunk(1)
            emit_conv(0, 1)
            emit_conv(1, 0)
            emit_predh(0)
            emit_predh(1)
            emit_loss(0)
            emit_out(0)
            emit_chunk(2)
            pab8 = constp.tile([1, 1], f32)
            pabs_sc = constp.tile([1, 128], f32)
            nc.scalar.activation(
                pabs_sc[:], pp_sb[:], AF.Abs, scale=0.125, accum_out=pab8[:]
            )
            emit_conv(1, 1)
            emit_conv(2, 0)
            emit_predh(2)
            emit_out(1)
            emit_predh(3)
            emit_loss(1)
            emit_out(2)
            emit_chunk(3)
            emit_conv(2, 1)
            emit_conv(3, 0)
            emit_predh(4)
            emit_out(3)
            emit_predh(5)
            emit_loss(2)
            emit_out(4)
            emit_conv(3, 1)
            emit_predh(6)
            emit_out(5)
            emit_predh(7)
            emit_loss(3)
            emit_out(6)
            emit_out(7)

            # ---- partition-reduce + finalize loss partial ------------------
            rp = rpsp.tile([1, 5], f32, tag="rp")
            nc.tensor.matmul(rp[:], ones[:], rstack[:], start=True, stop=True)
            s = constp.tile([1, 5], f32)
            smain = constp.tile([1, 1], f32)
            tmul = constp.tile([1, 1], f32)
            tsum = constp.tile([1, 1], f32)
            tscl = constp.tile([1, 1], f32)
            lsb = constp.tile([1, 1], f32)
            nc.vector.tensor_copy(s[:], rp[:])
            nc.vector.tensor_reduce(
                smain[:], s[:, 0:4], axis=mybir.AxisListType.X, op=ALU.add
            )
            nc.vector.tensor_mul(tmul[:], s[:, 4:5], fl_sb[:])
            nc.vector.tensor_add(tsum[:], smain[:], tmul[:])
            # lsb = tsum * (1/N) + 0.125 * sum|params|
            nc.vector.tensor_scalar_mul(tscl[:], tsum[:], INV_N)
            nc.vector.tensor_add(lsb[:], tscl[:], pab8[:])
            nc.sync.dma_start(lp[:, :], lsb[:])

    nc.compile()
    return nc


def _get_nc():
    import os

    key = ("nc", not os.environ.get("KERNEL_NO_F32R"))
    if key not in _CACHE:
        _CACHE[key] = _build_nc(use_f32r=key[1])
    return _CACHE[key]


def _host_inputs(y_serie, params):
    y = np.ascontiguousarray(np.asarray(y_serie, dtype=np.float32))
    p = np.ascontiguousarray(np.asarray(params, dtype=np.float32))
    assert y.shape == (N_FULL,) and p.shape == (P,)

    # W0[k, m] = params[k - m] for k >= m else 0
    # W1[k, m] = params[k + 128 - m] for k < m else 0
    # Both are strided windows into one zero-padded copy of params.
    qpad = np.zeros(383, dtype=np.float32)
    qpad[127:255] = p
    st = qpad.strides[0]
    w0 = np.ascontiguousarray(
        np.lib.stride_tricks.as_strided(qpad[127:], (128, 128), (st, -st))
    )
    w1 = np.ascontiguousarray(
        np.lib.stride_tricks.as_strided(qpad[255:], (128, 128), (st, -st))
    )
    prow = np.ascontiguousarray(p.reshape(1, 128))
    ident_np = np.eye(128, dtype=np.float32)

    in_maps = []
    for s in range(N_CORES):
        start = s * OWN
        in_maps.append(
            {
                "y": np.ascontiguousarray(y[start : start + SHARD_Y]),
                "w0": w0,
                "w1": w1,
                "pp": prow,
                "idin": ident_np,
                "fl": np.array(
                    [[1.0 if s == N_CORES - 1 else 0.0]], dtype=np.float32
                ),
            }
        )
    return in_maps


def _ensure_profile_hook():
    """Register the axon NTFF profile hook (missing antenv.axon_hooks shim)."""
    import sys
    import types

    try:
        import antenv

        if "antenv.axon_hooks" not in sys.modules:
            m = types.ModuleType("antenv.axon_hooks")
            m._hook = None

            def set_axon_ntff_profile_hook(hook):
                m._hook = hook

            def get_axon_ntff_profile_hook():
                return m._hook

            m.set_axon_ntff_profile_hook = set_axon_ntff_profile_hook
            m.get_axon_ntff_profile_hook = get_axon_ntff_profile_hook
            sys.modules["antenv.axon_hooks"] = m
            antenv.axon_hooks = m
        mod = sys.modules["antenv.axon_hooks"]
        if mod.get_axon_ntff_profile_hook() is None:
            from trn_agent_boot.trn_boot import _ntff_profile_via_ctypes

            hook = _ntff_profile_via_ctypes("/opt/axon/libaxon_pjrt.so")
            if hook is not None:
                mod.set_axon_ntff_profile_hook(hook)
    except Exception as e:  # profiling is best-effort
        print(f"profile hook setup failed: {e}", file=sys.stderr)


def kernel(y_serie, params, _trace=False):
    global LAST_EXEC_NS, LAST_RESULTS
    import os

    from concourse.bass_utils import run_bass_kernel_spmd

    trace = _trace or bool(os.environ.get("KERNEL_TRACE"))
    if trace:
        _ensure_profile_hook()
    in_maps = _host_inputs(y_serie, params)
    nc = _get_nc()
    res = run_bass_kernel_spmd(nc, in_maps, list(range(N_CORES)), trace=trace)
    LAST_EXEC_NS = res.exec_time_ns
    LAST_RESULTS = res

    pred = np.empty(N_PRED, dtype=np.float32)
    loss = np.float32(0.0)
    for s in range(N_CORES):
        out = res.results[s]
        take = SHARD_PRED if s == N_CORES - 1 else OWN
        pred[s * OWN : s * OWN + take] = out["pred"][:take]
        loss = np.float32(loss + out["lp"].reshape(()))
    return pred, loss


if __name__ == "__main__":
    rng = np.random.default_rng(0)
    y = rng.standard_normal(N_FULL).astype(np.float32)
    p = rng.standard_normal(P).astype(np.float32)
    pred, loss = kernel(y, p)
    print("pred[:4] =", pred[:4], "loss =", loss)


# revision 47
# speedup vs baseline: 1.1278x; 1.1278x over previous
"""AR(128) prediction + MSE/L1 loss on 8 Trainium2 NeuronCores.

pred[i] = sum_j params[j] * y[i+j]  (i = 0..1_999_999)  -- a 128-tap FIR.
loss    = mean((pred - y[128:])**2) + sum(|params|)

Strategy (per the row-shard + halo hint):
  * The series is split into 8 row-shards with a 128-sample halo.
    Core s computes preds [s*249984, s*249984 + 250112) (multiple-of-128
    shards that overlap by 128 so every core runs an identical program).
  * On device the FIR is phase-decomposed: with Yt[k, C] = y[128C + k],
      pred[128C + m] = sum_k W0[k, m] Yt[k, C] + sum_k W1[k, m] Yt[k, C+1]
    where W0[k, m] = params[k - m] (k >= m), W1[k, m] = params[k+128-m]
    (k < m).  W0/W1 are pure re-indexings of params (host as_strided,
    zero arithmetic), replicated to all cores.
  * y is DMA'd in 4 contiguous [128, F] chunks, transposed on-chip
    128x128-tile-wise by the tensor engine (chunk c covers exactly yt
    columns [512c, 512c+F)), the two banded matmuls accumulate bank c
    in PSUM, and the phase-major result is transposed back per chunk
    before contiguous [128, F] DMAs out.
  * The squared-error partial is computed per PSUM bank: the vector
    engine forms d = pred - y+ straight off PSUM, the scalar engine
    squares with a fused row-accumulate, and a ones-matmul
    partition-reduces.  Each core emits
    partial = sum(d^2)/2e6 + sum(|params|)/8; the host sums the 8
    scalars (a device AllReduce has a ~20us latency floor).
  * The conv matmuls run as float32r (single-pass reduced-precision
    fp32, ~1.5e-4 rel err vs the fp32 reference); set KERNEL_NO_F32R=1
    for exact fp32 (4 cycles/row instead of 1-2).
  * Dummy matmuls during the DMA-wait window plus real matmuls woven
    between the transpose groups keep the PE HAM activity monitor fed
    so the kernel runs at 2.4 GHz instead of the cold 1.2 GHz clock.
"""

import sys

if "/opt/trn_rl_repo" not in sys.path:
    sys.path.insert(0, "/opt/trn_rl_repo")

import numpy as np

P = 128
N_FULL = 2_000_128
N_PRED = 2_000_000
N_CORES = 8
OWN = 249_984          # preds owned by cores 0..6           (= 1953*128)
SHARD_PRED = 250_112   # preds computed per core             (= 1954*128)
SHARD_Y = 250_240      # y samples per core incl. halo       (= 1955*128)
NCOLS = 1954           # phase-major pred columns
YT_COLS = 1955         # transposed-y columns
INV_N = 1.0 / float(N_PRED)

# chunk table: (y/pred element offset, partition width F, 128x128 tiles)
CHUNKS = [(0, 512, 4), (65536, 512, 4), (131072, 512, 4), (196608, 384, 3)]

LAST_EXEC_NS = None
LAST_RESULTS = None

_CACHE = {}


def _build_nc(use_f32r=False):
    import concourse.bacc as bacc
    import concourse.tile as tile
    from concourse import mybir
    from concourse.masks import make_identity

    f32 = mybir.dt.float32
    f32r = mybir.dt.float32r
    AF = mybir.ActivationFunctionType
    ALU = mybir.AluOpType

    nc = bacc.Bacc("TRN2", target_bir_lowering=False, debug=False)

    y = nc.declare_dram_parameter("y", [SHARD_Y], f32, False)
    w0 = nc.declare_dram_parameter("w0", [128, 128], f32, False)
    w1 = nc.declare_dram_parameter("w1", [128, 128], f32, False)
    pp = nc.declare_dram_parameter("pp", [1, 128], f32, False)
    fl = nc.declare_dram_parameter("fl", [1, 1], f32, False)
    idin = nc.declare_dram_parameter("idin", [128, 128], f32, False)
    pred = nc.declare_dram_parameter("pred", [SHARD_PRED], f32, True)
    lp = nc.declare_dram_parameter("lp", [1, 1], f32, True)

    def mm_ap(ap):
        return ap.bitcast(f32r) if use_f32r else ap

    mm_dt = None  # set inside builder

    with tile.TileContext(nc) as tc:
        with (
            tc.tile_pool(name="const", bufs=1) as constp,
            tc.tile_pool(name="big", bufs=1) as bigp,
            tc.tile_pool(name="nbp", bufs=3) as nbp,
            tc.tile_pool(name="osbp", bufs=3) as osbp,
            tc.tile_pool(name="tps", bufs=3, space="PSUM") as tpsp,
            tc.tile_pool(name="mps", bufs=4, space="PSUM") as mpsp,
            tc.tile_pool(name="rps", bufs=1, space="PSUM") as rpsp,
        ):
            # ---- constants -------------------------------------------------
            w0_sb = constp.tile([128, 128], f32)
            w1_sb = constp.tile([128, 128], f32)
            pp_sb = constp.tile([1, 128], f32)
            fl_sb = constp.tile([1, 1], f32)
            mm_dt = f32r if use_f32r else f32
            ident = constp.tile([128, 128], mm_dt)
            ones = constp.tile([128, 1], f32)
            nc.gpsimd.memset(ones[:], 1.0)
            nc.scalar.dma_start(mm_ap(w0_sb[:]), mm_ap(w0[:, :]))
            nc.scalar.dma_start(ident[:], mm_ap(idin[:, :]))
            nc.scalar.dma_start(mm_ap(w1_sb[:]), mm_ap(w1[:, :]))

            yt = bigp.tile([128, YT_COLS], f32)
            nbtail = bigp.tile([35, 128], mm_dt)

            # rstack cols: 0-3 sum(d^2) main cols per bank, 4 tail col 1953
            rstack = constp.tile([128, 5], f32)
            df = bigp.tile([128, NCOLS], f32)
            sq = bigp.tile([128, NCOLS], f32)
            pred_sb = bigp.tile([128, NCOLS], f32)

            # HAM warm-up: the PE clock idles at 1.2 GHz and only ramps to
            # 2.4 GHz after ~3.4 us of sustained non-transpose matmul
            # activity.  Burn dummy matmuls on the identity during the
            # DMA-wait window so the real work runs at full clock.
            dumw = constp.tile([128, 128], f32)
            nc.vector.memset(dumw[:], 0.0)
            wps = rpsp.tile([128, 128], f32, tag="rp")
            for _ in range(8):
                nc.tensor.matmul(
                    wps[:], dumw[:], dumw[:], start=True, stop=True
                )

            ppsb = [None] * 4

            def emit_chunk(c):
                off, F, nt = CHUNKS[c]
                nbc = nbp.tile([128, 512], mm_dt, tag="nb")
                dma_eng = nc.sync if c % 2 == 0 else nc.scalar
                dma_eng.dma_start(
                    nbc[:, 0:F],
                    mm_ap(y[off : off + 128 * F].rearrange("(p f) -> p f", f=F)),
                )
                if c == 2:
                    nc.sync.dma_start(
                        nbtail[:],
                        mm_ap(y[245760:250240].rearrange("(p f) -> p f", f=128)),
                    )
                    nc.sync.dma_start(pp_sb[:], pp[:, :])
                    nc.sync.dma_start(fl_sb[:], fl[:, :])
                pst = tpsp.tile([128, 512], f32r if use_f32r else f32, tag="tps", name=f"pst{c}")
                for t in range(nt):
                    nc.tensor.transpose(
                        pst[:, 128 * t : 128 * (t + 1)],
                        nbc[:, 128 * t : 128 * (t + 1)],
                        ident[:],
                    )
                if c == 3:
                    nc.tensor.transpose(
                        pst[:, 384:419], nbtail[:], ident[0:35, 0:35]
                    )
                # pst[k, 128t + p] -> yt[k, off/128 + nt*p + t]
                ytv = yt[:, off // 128 : off // 128 + F].rearrange(
                    "k (p t) -> k p t", t=nt
                )
                psv = pst[:, 0 : 128 * nt].rearrange("k (t p) -> k p t", p=128)
                nc.vector.tensor_copy(mm_ap(ytv), psv)
                if c == 3:
                    nc.vector.tensor_copy(
                        mm_ap(yt[:, 1920:1955]), pst[:, 384:419]
                    )

            def emit_conv(b, h):
                # half-bank matmul pair: cols [256h, 256h+wh) of bank b
                c0 = 512 * b + 256 * h
                wb = 512 if b < 3 else NCOLS - 1536
                wh = min(256, wb - 256 * h)
                if h == 0:
                    ppsb[b] = mpsp.tile(
                        [128, 512], f32, tag="mm", name=f"ppsb{b}"
                    )
                o0 = 256 * h
                nc.tensor.matmul(
                    ppsb[b][:, o0 : o0 + wh],
                    mm_ap(w0_sb[:]),
                    mm_ap(yt[:, c0 : c0 + wh]),
                    start=True,
                    stop=False,
                )
                nc.tensor.matmul(
                    ppsb[b][:, o0 : o0 + wh],
                    mm_ap(w1_sb[:]),
                    mm_ap(yt[:, c0 + 1 : c0 + 1 + wh]),
                    start=False,
                    stop=True,
                )

            def emit_predh(hb):
                # copy one conv half-bank psum -> pred_sb (phase-major)
                b, h = hb // 2, hb % 2
                wb = 512 if b < 3 else NCOLS - 1536
                wh = min(256, wb - 256 * h)
                c0 = 512 * b + 256 * h
                nc.vector.tensor_copy(
                    pred_sb[:, c0 : c0 + wh], ppsb[b][:, 256 * h : 256 * h + wh]
                )

            def emit_loss(b):
                # DVE: diff; ACT: square + row-accum
                c0 = 512 * b
                w = 512 if b < 3 else NCOLS - 1536
                nc.vector.tensor_sub(
                    df[:, c0 : c0 + w],
                    ppsb[b][:, 0:w],
                    yt[:, c0 + 1 : c0 + 1 + w],
                )
                wm = w if b < 3 else w - 1  # main cols exclude col 1953
                nc.scalar.activation(
                    sq[:, c0 : c0 + wm],
                    df[:, c0 : c0 + wm],
                    AF.Square,
                    accum_out=rstack[:, b : b + 1],
                )
                if b == 3:
                    nc.scalar.activation(
                        sq[:, 1953:1954],
                        df[:, 1953:1954],
                        AF.Square,
                        accum_out=rstack[:, 4:5],
                    )

            def emit_out(hb):
                # transpose one half-bank of pred back to natural + DMA out.
                # out-chunk hb covers pred cols [256*hb, 256*hb + F).
                F = 256 if hb < 7 else 128
                nt = F // 128
                off = 32768 * hb
                c0 = 256 * hb
                ops = tpsp.tile([128, 512], f32, tag="tps")
                pmv = pred_sb[:, c0 : c0 + F].rearrange("m (p t) -> m p t", t=nt)
                for t in range(nt):
                    nc.tensor.transpose(
                        ops[:, 128 * t : 128 * (t + 1)], pmv[:, :, t], ident[:]
                    )
                if hb == 7:
                    nc.tensor.transpose(
                        ops[0:34, 128:256], pred_sb[:, 1920:1954], ident[:]
                    )
                osb = osbp.tile([128, 512], f32, tag="osb")
                if hb % 2 == 0:
                    nc.scalar.copy(osb[:, 0:F], ops[:, 0:F])
                else:
                    nc.vector.tensor_copy(osb[:, 0:F], ops[:, 0:F])
                dma_eng = nc.sync if hb % 2 == 0 else nc.scalar
                dma_eng.dma_start(
                    pred[off : off + 128 * F].rearrange("(p f) -> p f", f=F),
                    osb[:, 0:F],
                )
                if hb == 7:
                    nc.vector.tensor_copy(osb[0:34, 128:256], ops[0:34, 128:256])
                    nc.sync.dma_start(
                        pred[245760:250112].rearrange("(p f) -> p f", f=128),
                        osb[0:34, 128:256],
                    )

            # pipelined emission; conv half (b,0) needs chunk b, (b,1) needs
            # chunks b and b+1 (halo column).  Real matmuls and output
            # transposes woven between the transpose groups keep the HAM
            # activity monitor fed and spread the output DMAs early.
            emit_chunk(0)
            emit_conv(0, 0)
            emit_chunk(1)
            emit_conv(0, 1)
            emit_conv(1, 0)
            emit_predh(0)
            emit_predh(1)
            emit_loss(0)
            emit_out(0)
            emit_chunk(2)
            pab8 = constp.tile([1, 1], f32)
            pabs_sc = constp.tile([1, 128], f32)
            nc.scalar.activation(
                pabs_sc[:], pp_sb[:], AF.Abs, scale=0.125, accum_out=pab8[:]
            )
            emit_conv(1, 1)
            emit_conv(2, 0)
            emit_predh(2)
            emit_out(1)
            emit_predh(3)
            emit_loss(1)
            emit_out(2)
            emit_chunk(3)
            emit_conv(2, 1)
            emit_conv(3, 0)
            emit_predh(4)
            emit_out(3)
            emit_predh(5)
            emit_loss(2)
            emit_out(4)
            emit_conv(3, 1)
            emit_predh(6)
            emit_out(5)
            emit_predh(7)
            emit_loss(3)
            emit_out(6)
            emit_out(7)

            # ---- partition-reduce + finalize loss partial ------------------
            rp = rpsp.tile([1, 5], f32, tag="rp")
            nc.tensor.matmul(rp[:], ones[:], rstack[:], start=True, stop=True)
            s = constp.tile([1, 5], f32)
            smain = constp.tile([1, 1], f32)
            tmul = constp.tile([1, 1], f32)
            tsum = constp.tile([1, 1], f32)
            tscl = constp.tile([1, 1], f32)
            lsb = constp.tile([1, 1], f32)
            nc.vector.tensor_copy(s[:], rp[:])
            nc.vector.tensor_reduce(
                smain[:], s[:, 0:4], axis=mybir.AxisListType.X, op=ALU.add
            )
            nc.vector.tensor_mul(tmul[:], s[:, 4:5], fl_sb[:])
            nc.vector.tensor_add(tsum[:], smain[:], tmul[:])
            # lsb = tsum * (1/N) + 0.125 * sum|params|
            nc.vector.tensor_scalar_mul(tscl[:], tsum[:], INV_N)
            nc.vector.tensor_add(lsb[:], tscl[:], pab8[:])
            nc.sync.dma_start(lp[:, :], lsb[:])

    nc.compile()
    return nc


def _get_nc():
    import os

    key = ("nc", not os.environ.get("KERNEL_NO_F32R"))
    if key not in _CACHE:
        _CACHE[key] = _build_nc(use_f32r=key[1])
    return _CACHE[key]


def _host_inputs(y_serie, params):
    y = np.ascontiguousarray(np.asarray(y_serie, dtype=np.float32))
    p = np.ascontiguousarray(np.asarray(params, dtype=np.float32))
    assert y.shape == (N_FULL,) and p.shape == (P,)

    # W0[k, m] = params[k - m] for k >= m else 0
    # W1[k, m] = params[k + 128 - m] for k < m else 0
    # Both are strided windows into one zero-padded copy of params.
    qpad = np.zeros(383, dtype=np.float32)
    qpad[127:255] = p
    st = qpad.strides[0]
    w0 = np.ascontiguousarray(
        np.lib.stride_tricks.as_strided(qpad[127:], (128, 128), (st, -st))
    )
    w1 = np.ascontiguousarray(
        np.lib.stride_tricks.as_strided(qpad[255:], (128, 128), (st, -st))
    )
    prow = np.ascontiguousarray(p.reshape(1, 128))
    ident_np = np.eye(128, dtype=np.float32)

    in_maps = []
    for s in range(N_CORES):
        start = s * OWN
        in_maps.append(
            {
                "y": np.ascontiguousarray(y[start : start + SHARD_Y]),
                "w0": w0,
                "w1": w1,
                "pp": prow,
                "idin": ident_np,
                "fl": np.array(
                    [[1.0 if s == N_CORES - 1 else 0.0]], dtype=np.float32
                ),
            }
        )
    return in_maps


def _ensure_profile_hook():
    """Register the axon NTFF profile hook (missing antenv.axon_hooks shim)."""
    import sys
    import types

    try:
        import antenv

        if "antenv.axon_hooks" not in sys.modules:
            m = types.ModuleType("antenv.axon_hooks")
            m._hook = None

            def set_axon_ntff_profile_hook(hook):
                m._hook = hook

            def get_axon_ntff_profile_hook():
                return m._hook

            m.set_axon_ntff_profile_hook = set_axon_ntff_profile_hook
            m.get_axon_ntff_profile_hook = get_axon_ntff_profile_hook
            sys.modules["antenv.axon_hooks"] = m
            antenv.axon_hooks = m
        mod = sys.modules["antenv.axon_hooks"]
        if mod.get_axon_ntff_profile_hook() is None:
            from trn_agent_boot.trn_boot import _ntff_profile_via_ctypes

            hook = _ntff_profile_via_ctypes("/opt/axon/libaxon_pjrt.so")
            if hook is not None:
                mod.set_axon_ntff_profile_hook(hook)
    except Exception as e:  # profiling is best-effort
        print(f"profile hook setup failed: {e}", file=sys.stderr)


def kernel(y_serie, params, _trace=False):
    global LAST_EXEC_NS, LAST_RESULTS
    import os

    from concourse.bass_utils import run_bass_kernel_spmd

    trace = _trace or bool(os.environ.get("KERNEL_TRACE"))
    if trace:
        _ensure_profile_hook()
    in_maps = _host_inputs(y_serie, params)
    nc = _get_nc()
    res = run_bass_kernel_spmd(nc, in_maps, list(range(N_CORES)), trace=trace)
    LAST_EXEC_NS = res.exec_time_ns
    LAST_RESULTS = res

    pred = np.empty(N_PRED, dtype=np.float32)
    loss = np.float32(0.0)
    for s in range(N_CORES):
        out = res.results[s]
        take = SHARD_PRED if s == N_CORES - 1 else OWN
        pred[s * OWN : s * OWN + take] = out["pred"][:take]
        loss = np.float32(loss + out["lp"].reshape(()))
    return pred, loss


if __name__ == "__main__":
    rng = np.random.default_rng(0)
    y = rng.standard_normal(N_FULL).astype(np.float32)
    p = rng.standard_normal(P).astype(np.float32)
    pred, loss = kernel(y, p)
    print("pred[:4] =", pred[:4], "loss =", loss)
